# revision 18
# baseline (speedup 1.0000x reference)
"""Trainium2 Bass kernel for nn_DeepPolyConv2D.

Materializes the dense linear map (W, b) of a stride-2 / pad-1 conv2d over a
(1, 3, 48, 48) input, as the reference does via conv-over-identity:

    W[(o,i,j), (c,h,w)] = K[o, c, h-2i+1, w-2j+1]   when both kernel indices
                                                     are in [0,4), else 0
    b[(o,i,j)]          = bias[o]

Sharding: output rows (o,i,j) split across 8 cores — core k owns output
channels {2k, 2k+1}, i.e. rows [k*1152, (k+1)*1152) of W.

Device-side strategy (memory-regime problem; ~31.9 MB of mostly-zero f32
output per core): for a fixed (o, i), every row's nonzeros within channel c
live in one contiguous 768 B "strip" (image rows h = 2i-1 .. 2i+2, full
width) whose position inside the row depends only on (i, c) and whose content
depends only on (o, c, j).  So the per-core output, viewed as 3456
channel-thirds of 9216 B, is processed in 27 blocks of 128 thirds held in
SBUF [128, 2304] f32 tiles: the tiles stay all-zero except the strip bytes,
which are DMA-loaded from a small host-precomputed table; each tile is then
written to DRAM with a single fully-contiguous 1.18 MB DMA, and the strip
bytes are re-zeroed for the tile's next use.  Every output byte is written
exactly once by large contiguous DMAs, so the kernel runs at HBM write
bandwidth.
"""

import sys

for _p in ("/opt/trn_rl_repo",):
    if _p not in sys.path:
        sys.path.append(_p)

import numpy as np

# ---- fixed problem geometry (hardcoded per spec) ----
O, C, KH, KW = 16, 3, 4, 4
H = W = 48
STRIDE, PAD = 2, 1
Ho = Wo = 24
N = C * H * W              # 6912 input coords
R = O * Ho * Wo            # 9216 output rows
NCORES = 8
RPC = R // NCORES          # 1152 rows per core
THIRD = H * W              # 2304 elems per (row, channel) third
NTHIRD = RPC * C           # 3456 thirds per core
NBLK = NTHIRD // 128       # 27 blocks of 128 thirds
SLEN = KH * W              # 192 elems: max strip length
NBUF = 8


def _strip_params(i):
    """(offset_elems, length_elems, first_kernel_row) of row-group i's strip."""
    if i == 0:
        return 0, 3 * W, 1
    if i == Ho - 1:
        return (2 * i - 1) * W, 3 * W, 0
    return (2 * i - 1) * W, 4 * W, 0


def _segments(T):
    """Maximal runs of constant i within block T: list of (p0, p1, off, L)."""
    segs = []
    p = 0
    while p < 128:
        g = 128 * T + p
        i = (g // (C * Wo)) % Ho
        ln = min(C * Wo - (g % (C * Wo)), 128 - p)
        off, L, _ = _strip_params(i)
        segs.append((p, p + ln, off, L))
        p += ln
    return segs


# Per block, segments cluster into contiguous column windows (consecutive i
# offsets differ by 96 < 192, except at the o-boundary where i wraps 23 -> 0).
PATCHW = 384  # ≤3 i-values per contiguous window -> ≤ 2*96 + 192


def _windows(T):
    """Cluster block T's segments into windows: [(p0, p1, lo, width), ...]."""
    segs = _segments(T)
    clusters = []
    cur = [segs[0]]
    for s in segs[1:]:
        if abs(s[2] - cur[-1][2]) <= SLEN:
            cur.append(s)
        else:
            clusters.append(cur)
            cur = [s]
    clusters.append(cur)
    wins = []
    for cl in clusters:
        p0, p1 = cl[0][0], cl[-1][1]
        lo = min(off for _, _, off, _ in cl)
        hi = max(off + L for _, _, off, L in cl)
        assert p0 % 32 == 0, (T, p0)
        assert hi - lo <= PATCHW, (T, lo, hi)
        wins.append((p0, p1, lo, hi - lo, cl))
    return wins


def _build_utab(kern):
    """U[core][72, 2*SLEN] f32: the 72 unique interior strip rows per o.

    Row k = 3j + c, columns [o_local*SLEN + u*48 + w] hold
    K[2*core + o_local, c, u, w - 2j + 1] (masked to valid kernel cols).
    Edge strips (i = 0 / 23) are column subranges [48:192] / [0:144].
    """
    # SC[o, c, j, u, w] = K[o, c, u, w - 2j + 1] (0 where out of range)
    j_idx = np.arange(Wo)
    w_idx = np.arange(W)
    v = w_idx[None, :] - (2 * j_idx[:, None] - 1)          # [j, w]
    valid = (v >= 0) & (v < KW)
    vc = np.clip(v, 0, KW - 1)
    # K[o, c, u, vc[j, w]] -> [o, c, u, j, w] -> [o, j, c, u, w]
    SC = kern[:, :, :, vc].transpose(0, 3, 1, 2, 4) * valid[None, :, None, None, :]
    SC = np.ascontiguousarray(SC, dtype=np.float32)        # [O, Wo, C, KH, W]
    out = []
    for core in range(NCORES):
        u = SC[2 * core:2 * core + 2]                       # [2, Wo, C, KH, W]
        u = u.reshape(2, 72, SLEN).transpose(1, 0, 2).reshape(72, 2 * SLEN)
        out.append(np.ascontiguousarray(u))
    return out


_PROGRAM = None


def _get_program():
    global _PROGRAM
    if _PROGRAM is not None:
        return _PROGRAM

    import concourse.bacc as bacc
    import concourse.mybir as mybir
    from concourse.ap import AP
    from concourse.tile import TileContext

    f32 = mybir.dt.float32
    nc = bacc.Bacc("TRN2", target_bir_lowering=False)
    u_tab = nc.dram_tensor("u_tab", [72, 2 * SLEN], f32, kind="ExternalInput")
    bvec = nc.dram_tensor("bvec", [RPC], f32, kind="ExternalInput")
    w_out = nc.dram_tensor("w_out", [RPC, N], f32, kind="ExternalOutput")
    b_out = nc.dram_tensor("b_out", [RPC], f32, kind="ExternalOutput")

    with TileContext(nc) as tc:
        with tc.tile_pool(name="bufs", bufs=1) as pool:
            bufs = [
                pool.tile([128, THIRD], f32, name=f"buf{n}", tag=f"buf{n}")
                for n in range(NBUF)
            ]
            # resident strip-row table (110 KB, loaded once)
            u_sb = pool.tile([72, 2 * SLEN], f32, name="u_sb", tag="u_sb")
            nc.sync.dma_start(out=u_sb[:, :], in_=u_tab[:, :])
            # write buffers start all-zero (split across DVE / GpSimd)
            for n, b in enumerate(bufs):
                eng = nc.vector if n % 2 == 0 else nc.gpsimd
                eng.memset(b[:], 0.0)
            # bias: plain DRAM->DRAM copy (4.6 KB)
            nc.scalar.dma_start(out=b_out[:], in_=bvec[:])
            for T in range(NBLK):
                buf = bufs[T % NBUF]
                ring_w = nc.sync
                ring_s = nc.scalar
                wins = _windows(T)
                # stitch strips into the otherwise-zero tile: per-segment
                # SBUF->SBUF DMAs (DMA has no start-partition restriction;
                # within a segment k = (128T + p) mod 72 never wraps)
                for (_, _, _, _, cl) in wins:
                    for (p0, p1, off, L) in cl:
                        u0 = 1 if (off == 0 and L == 3 * W) else 0
                        o_l = (128 * T + p0) // (576 * C)
                        k0 = (128 * T + p0) % 72
                        col = o_l * SLEN + u0 * W
                        ring_s.dma_start(
                            out=buf[p0:p1, off:off + L],
                            in_=u_sb[k0:k0 + (p1 - p0), col:col + L],
                        )
                # one fully-contiguous 1.18 MB write; alternate HWDGE rings
                dst = AP(w_out, T * 128 * THIRD, [[1, 128 * THIRD]])
                ring_w.dma_start(out=dst, in_=buf[:, :])
                # restore all-zeros for this tile's next use (window-wise,
                # quadrant-aligned partition ranges for the compute engines)
                if T + NBUF < NBLK:
                    eng = nc.vector if T % 2 == 0 else nc.gpsimd
                    for (p0w, p1w, lo, width, _) in wins:
                        eng.memset(buf[p0w:p1w, lo:lo + width], 0.0)

    nc.finalize()
    _PROGRAM = nc
    return nc


# test.py hooks: set TRACE=True before calling kernel() to profile; the
# BassKernelResults of the last run lands in LAST_RESULTS.
TRACE = False
TRACE_KWARGS = {}
LAST_RESULTS = None


def kernel(**inputs):
    from concourse.bass_utils import run_bass_kernel_spmd

    kern = np.asarray(inputs["kernel"], dtype=np.float32)
    bias = np.asarray(inputs["bias"], dtype=np.float32)
    stride = int(inputs.get("stride", STRIDE))
    padding = int(inputs.get("padding", PAD))
    assert kern.shape == (O, C, KH, KW), kern.shape
    assert stride == STRIDE and padding == PAD, (stride, padding)

    utab = _build_utab(kern)
    in_maps = [
        {
            "u_tab": utab[core],
            "bvec": np.repeat(bias[2 * core:2 * core + 2], Ho * Wo),
        }
        for core in range(NCORES)
    ]
    for m in in_maps:
        assert m["bvec"].shape == (RPC,), m["bvec"].shape

    nc = _get_program()
    res = run_bass_kernel_spmd(
        nc,
        in_maps,
        core_ids=list(range(NCORES)),
        trace=TRACE,
        **TRACE_KWARGS,
    )
    global LAST_RESULTS
    LAST_RESULTS = res

    Wm = np.concatenate([res.results[c]["w_out"] for c in range(NCORES)], axis=0)
    bm = np.concatenate([res.results[c]["b_out"] for c in range(NCORES)], axis=0)
    return Wm, bm


# revision 20
# speedup vs baseline: 1.0234x; 1.0234x over previous
"""Trainium2 Bass kernel for nn_DeepPolyConv2D.

Materializes the dense linear map (W, b) of a stride-2 / pad-1 conv2d over a
(1, 3, 48, 48) input, as the reference does via conv-over-identity:

    W[(o,i,j), (c,h,w)] = K[o, c, h-2i+1, w-2j+1]   when both kernel indices
                                                     are in [0,4), else 0
    b[(o,i,j)]          = bias[o]

Sharding: output rows (o,i,j) split across 8 cores — core k owns output
channels {2k, 2k+1}, i.e. rows [k*1152, (k+1)*1152) of W.

Device-side strategy (memory-regime problem; ~31.9 MB of mostly-zero f32
output per core): for a fixed (o, i), every row's nonzeros within channel c
live in one contiguous 768 B "strip" (image rows h = 2i-1 .. 2i+2, full
width) whose position inside the row depends only on (i, c) and whose content
depends only on (o, c, j).  So the per-core output, viewed as 3456
channel-thirds of 9216 B, is processed in 27 blocks of 128 thirds held in
SBUF [128, 2304] f32 tiles: the tiles stay all-zero except the strip bytes,
which are DMA-loaded from a small host-precomputed table; each tile is then
written to DRAM with a single fully-contiguous 1.18 MB DMA, and the strip
bytes are re-zeroed for the tile's next use.  Every output byte is written
exactly once by large contiguous DMAs, so the kernel runs at HBM write
bandwidth.
"""

import sys

for _p in ("/opt/trn_rl_repo",):
    if _p not in sys.path:
        sys.path.append(_p)

import numpy as np

# ---- fixed problem geometry (hardcoded per spec) ----
O, C, KH, KW = 16, 3, 4, 4
H = W = 48
STRIDE, PAD = 2, 1
Ho = Wo = 24
N = C * H * W              # 6912 input coords
R = O * Ho * Wo            # 9216 output rows
NCORES = 8
RPC = R // NCORES          # 1152 rows per core
THIRD = H * W              # 2304 elems per (row, channel) third
NTHIRD = RPC * C           # 3456 thirds per core
NBLK = NTHIRD // 128       # 27 blocks of 128 thirds
SLEN = KH * W              # 192 elems: max strip length
NBUF = 8


def _strip_params(i):
    """(offset_elems, length_elems, first_kernel_row) of row-group i's strip."""
    if i == 0:
        return 0, 3 * W, 1
    if i == Ho - 1:
        return (2 * i - 1) * W, 3 * W, 0
    return (2 * i - 1) * W, 4 * W, 0


def _segments(T):
    """Maximal runs of constant i within block T: list of (p0, p1, off, L)."""
    segs = []
    p = 0
    while p < 128:
        g = 128 * T + p
        i = (g // (C * Wo)) % Ho
        ln = min(C * Wo - (g % (C * Wo)), 128 - p)
        off, L, _ = _strip_params(i)
        segs.append((p, p + ln, off, L))
        p += ln
    return segs


# Per block, segments cluster into contiguous column windows (consecutive i
# offsets differ by 96 < 192, except at the o-boundary where i wraps 23 -> 0).
PATCHW = 384  # ≤3 i-values per contiguous window -> ≤ 2*96 + 192


def _windows(T):
    """Cluster block T's segments into windows: [(p0, p1, lo, width), ...]."""
    segs = _segments(T)
    clusters = []
    cur = [segs[0]]
    for s in segs[1:]:
        if abs(s[2] - cur[-1][2]) <= SLEN:
            cur.append(s)
        else:
            clusters.append(cur)
            cur = [s]
    clusters.append(cur)
    wins = []
    for cl in clusters:
        p0, p1 = cl[0][0], cl[-1][1]
        lo = min(off for _, _, off, _ in cl)
        hi = max(off + L for _, _, off, L in cl)
        assert p0 % 32 == 0, (T, p0)
        assert hi - lo <= PATCHW, (T, lo, hi)
        wins.append((p0, p1, lo, hi - lo, cl))
    return wins


def _build_utab(kern):
    """U[core][72, 2*SLEN] f32: the 72 unique interior strip rows per o.

    Row k = 3j + c, columns [o_local*SLEN + u*48 + w] hold
    K[2*core + o_local, c, u, w - 2j + 1] (masked to valid kernel cols).
    Edge strips (i = 0 / 23) are column subranges [48:192] / [0:144].
    """
    # SC[o, c, j, u, w] = K[o, c, u, w - 2j + 1] (0 where out of range)
    j_idx = np.arange(Wo)
    w_idx = np.arange(W)
    v = w_idx[None, :] - (2 * j_idx[:, None] - 1)          # [j, w]
    valid = (v >= 0) & (v < KW)
    vc = np.clip(v, 0, KW - 1)
    # K[o, c, u, vc[j, w]] -> [o, c, u, j, w] -> [o, j, c, u, w]
    SC = kern[:, :, :, vc].transpose(0, 3, 1, 2, 4) * valid[None, :, None, None, :]
    SC = np.ascontiguousarray(SC, dtype=np.float32)        # [O, Wo, C, KH, W]
    out = []
    for core in range(NCORES):
        u = SC[2 * core:2 * core + 2]                       # [2, Wo, C, KH, W]
        u = u.reshape(2, 72, SLEN).transpose(1, 0, 2).reshape(72, 2 * SLEN)
        out.append(np.ascontiguousarray(u))
    return out


_PROGRAM = None


def _get_program():
    global _PROGRAM
    if _PROGRAM is not None:
        return _PROGRAM

    import concourse.bacc as bacc
    import concourse.mybir as mybir
    from concourse.ap import AP
    from concourse.tile import TileContext

    f32 = mybir.dt.float32
    nc = bacc.Bacc("TRN2", target_bir_lowering=False)
    u_tab = nc.dram_tensor("u_tab", [72, 2 * SLEN], f32, kind="ExternalInput")
    bvec = nc.dram_tensor("bvec", [RPC], f32, kind="ExternalInput")
    w_out = nc.dram_tensor("w_out", [RPC, N], f32, kind="ExternalOutput")
    b_out = nc.dram_tensor("b_out", [RPC], f32, kind="ExternalOutput")

    with TileContext(nc) as tc:
        with tc.tile_pool(name="bufs", bufs=1) as pool:
            bufs = [
                pool.tile([128, THIRD], f32, name=f"buf{n}", tag=f"buf{n}")
                for n in range(NBUF)
            ]
            # resident strip-row table (110 KB, loaded once)
            u_sb = pool.tile([72, 2 * SLEN], f32, name="u_sb", tag="u_sb")
            nc.sync.dma_start(out=u_sb[:, :], in_=u_tab[:, :])
            # write buffers start all-zero (split across DVE / GpSimd)
            for n, b in enumerate(bufs):
                eng = nc.vector if n % 2 == 0 else nc.gpsimd
                eng.memset(b[:], 0.0)
            # bias: plain DRAM->DRAM copy (4.6 KB)
            nc.scalar.dma_start(out=b_out[:], in_=bvec[:])
            for T in range(NBLK):
                buf = bufs[T % NBUF]
                ring_w = nc.sync if T % 2 == 0 else nc.scalar
                ring_s = nc.gpsimd  # SWDGE: keep HWDGE rings free for writes
                wins = _windows(T)
                # stitch strips into the otherwise-zero tile: per-segment
                # SBUF->SBUF DMAs (DMA has no start-partition restriction;
                # within a segment k = (128T + p) mod 72 never wraps)
                for (_, _, _, _, cl) in wins:
                    for (p0, p1, off, L) in cl:
                        u0 = 1 if (off == 0 and L == 3 * W) else 0
                        o_l = (128 * T + p0) // (576 * C)
                        k0 = (128 * T + p0) % 72
                        col = o_l * SLEN + u0 * W
                        ring_s.dma_start(
                            out=buf[p0:p1, off:off + L],
                            in_=u_sb[k0:k0 + (p1 - p0), col:col + L],
                        )
                # one fully-contiguous 1.18 MB write; alternate HWDGE rings
                dst = AP(w_out, T * 128 * THIRD, [[1, 128 * THIRD]])
                ring_w.dma_start(out=dst, in_=buf[:, :])
                # restore all-zeros for this tile's next use (window-wise,
                # quadrant-aligned partition ranges; DVE, which is idle)
                if T + NBUF < NBLK:
                    for (p0w, p1w, lo, width, _) in wins:
                        nc.vector.memset(buf[p0w:p1w, lo:lo + width], 0.0)

    nc.finalize()
    _PROGRAM = nc
    return nc


# test.py hooks: set TRACE=True before calling kernel() to profile; the
# BassKernelResults of the last run lands in LAST_RESULTS.
TRACE = False
TRACE_KWARGS = {}
LAST_RESULTS = None


def kernel(**inputs):
    from concourse.bass_utils import run_bass_kernel_spmd

    kern = np.asarray(inputs["kernel"], dtype=np.float32)
    bias = np.asarray(inputs["bias"], dtype=np.float32)
    stride = int(inputs.get("stride", STRIDE))
    padding = int(inputs.get("padding", PAD))
    assert kern.shape == (O, C, KH, KW), kern.shape
    assert stride == STRIDE and padding == PAD, (stride, padding)

    utab = _build_utab(kern)
    in_maps = [
        {
            "u_tab": utab[core],
            "bvec": np.repeat(bias[2 * core:2 * core + 2], Ho * Wo),
        }
        for core in range(NCORES)
    ]
    for m in in_maps:
        assert m["bvec"].shape == (RPC,), m["bvec"].shape

    nc = _get_program()
    res = run_bass_kernel_spmd(
        nc,
        in_maps,
        core_ids=list(range(NCORES)),
        trace=TRACE,
        **TRACE_KWARGS,
    )
    global LAST_RESULTS
    LAST_RESULTS = res

    Wm = np.concatenate([res.results[c]["w_out"] for c in range(NCORES)], axis=0)
    bm = np.concatenate([res.results[c]["b_out"] for c in range(NCORES)], axis=0)
    return Wm, bm


# revision 23
# speedup vs baseline: 1.0379x; 1.0141x over previous
"""Trainium2 Bass kernel for nn_DeepPolyConv2D.

Materializes the dense linear map (W, b) of a stride-2 / pad-1 conv2d over a
(1, 3, 48, 48) input, as the reference does via conv-over-identity:

    W[(o,i,j), (c,h,w)] = K[o, c, h-2i+1, w-2j+1]   when both kernel indices
                                                     are in [0,4), else 0
    b[(o,i,j)]          = bias[o]

Sharding: output rows (o,i,j) split across 8 cores — core k owns output
channels {2k, 2k+1}, i.e. rows [k*1152, (k+1)*1152) of W.

Device-side strategy (memory-regime problem; ~31.9 MB of mostly-zero f32
output per core): for a fixed (o, i), every row's nonzeros within channel c
live in one contiguous 768 B "strip" (image rows h = 2i-1 .. 2i+2, full
width) whose position inside the row depends only on (i, c) and whose content
depends only on (o, c, j).  So the per-core output, viewed as 3456
channel-thirds of 9216 B, is processed in 27 blocks of 128 thirds held in
SBUF [128, 2304] f32 tiles: the tiles stay all-zero except the strip bytes,
which are DMA-loaded from a small host-precomputed table; each tile is then
written to DRAM with a single fully-contiguous 1.18 MB DMA, and the strip
bytes are re-zeroed for the tile's next use.  Every output byte is written
exactly once by large contiguous DMAs, so the kernel runs at HBM write
bandwidth.
"""

import sys

for _p in ("/opt/trn_rl_repo",):
    if _p not in sys.path:
        sys.path.append(_p)

import numpy as np

# ---- fixed problem geometry (hardcoded per spec) ----
O, C, KH, KW = 16, 3, 4, 4
H = W = 48
STRIDE, PAD = 2, 1
Ho = Wo = 24
N = C * H * W              # 6912 input coords
R = O * Ho * Wo            # 9216 output rows
NCORES = 8
RPC = R // NCORES          # 1152 rows per core
THIRD = H * W              # 2304 elems per (row, channel) third
NTHIRD = RPC * C           # 3456 thirds per core
NBLK = NTHIRD // 128       # 27 blocks of 128 thirds
SLEN = KH * W              # 192 elems: max strip length
NBUF = 8


def _strip_params(i):
    """(offset_elems, length_elems, first_kernel_row) of row-group i's strip."""
    if i == 0:
        return 0, 3 * W, 1
    if i == Ho - 1:
        return (2 * i - 1) * W, 3 * W, 0
    return (2 * i - 1) * W, 4 * W, 0


def _segments(T):
    """Maximal runs of constant i within block T: list of (p0, p1, off, L)."""
    segs = []
    p = 0
    while p < 128:
        g = 128 * T + p
        i = (g // (C * Wo)) % Ho
        ln = min(C * Wo - (g % (C * Wo)), 128 - p)
        off, L, _ = _strip_params(i)
        segs.append((p, p + ln, off, L))
        p += ln
    return segs


# Per block, segments cluster into contiguous column windows (consecutive i
# offsets differ by 96 < 192, except at the o-boundary where i wraps 23 -> 0).
PATCHW = 384  # ≤3 i-values per contiguous window -> ≤ 2*96 + 192


def _windows(T):
    """Cluster block T's segments into windows: [(p0, p1, lo, width), ...]."""
    segs = _segments(T)
    clusters = []
    cur = [segs[0]]
    for s in segs[1:]:
        if abs(s[2] - cur[-1][2]) <= SLEN:
            cur.append(s)
        else:
            clusters.append(cur)
            cur = [s]
    clusters.append(cur)
    wins = []
    for cl in clusters:
        p0, p1 = cl[0][0], cl[-1][1]
        lo = min(off for _, _, off, _ in cl)
        hi = max(off + L for _, _, off, L in cl)
        assert p0 % 32 == 0, (T, p0)
        assert hi - lo <= PATCHW, (T, lo, hi)
        wins.append((p0, p1, lo, hi - lo, cl))
    return wins


# packed patch-table column layout: window w of block T lives at columns
# [_PCOL[(T, wi)], +width); total packed width _PTW
_PCOL = {}
_PTW = 0
for _T in range(NBLK):
    for _wi, (_p0, _p1, _lo, _wd, _cl) in enumerate(_windows(_T)):
        _PCOL[(_T, _wi)] = _PTW
        _PTW += _wd


def _build_strips(kern):
    """Packed patch table [128, _PTW] f32 per core.

    Third g (within a core) -> row = g//3, c = g%3, o = 2*core + row//576,
    i = (row%576)//24, j = row%24.  Strip floats at [u*48 + w] are
    K[o, c, u0+u, w-2j+1] (masked to valid kernel cols); block T's strips
    are stored window-relative at the packed column offsets _PCOL.
    """
    # SC[o, c, j, u, w] = K[o, c, u, w - 2j + 1] (0 where out of range)
    j_idx = np.arange(Wo)
    w_idx = np.arange(W)
    v = w_idx[None, :] - (2 * j_idx[:, None] - 1)          # [j, w]
    valid = (v >= 0) & (v < KW)
    vc = np.clip(v, 0, KW - 1)
    # K[o, c, u, vc[j, w]] -> [o, c, u, j, w] -> [o, c, j, u, w]
    SC = kern[:, :, :, vc].transpose(0, 1, 3, 2, 4) * valid[None, None, :, None, :]
    SC = np.ascontiguousarray(SC, dtype=np.float32)        # [O, C, Wo, KH, W]

    g = np.arange(NTHIRD)
    row = g // C
    c = g % C
    i = (row % (Ho * Wo)) // Wo
    j = row % Wo
    m_top = i == 0
    m_bot = i == Ho - 1
    m_int = ~(m_top | m_bot)

    out = []
    for core in range(NCORES):
        o = 2 * core + row // (Ho * Wo)
        base = SC[o, c, j]                                  # [NTHIRD, KH, W]
        res = np.zeros((NTHIRD, SLEN), dtype=np.float32)
        res[m_int] = base[m_int].reshape(-1, SLEN)
        res[m_top, : 3 * W] = base[m_top][:, 1:4, :].reshape(-1, 3 * W)
        res[m_bot, : 3 * W] = base[m_bot][:, 0:3, :].reshape(-1, 3 * W)
        res = res.reshape(NBLK, 128, SLEN)
        packed = np.zeros((128, _PTW), dtype=np.float32)
        for T in range(NBLK):
            for wi, (_, _, lo, _, cl) in enumerate(_windows(T)):
                pc = _PCOL[(T, wi)]
                for (p0, p1, off, L) in cl:
                    packed[p0:p1, pc + off - lo:pc + off - lo + L] = \
                        res[T, p0:p1, :L]
        out.append(packed)
    return out


_PROGRAM = None


def _get_program():
    global _PROGRAM
    if _PROGRAM is not None:
        return _PROGRAM

    import concourse.bacc as bacc
    import concourse.mybir as mybir
    from concourse.ap import AP
    from concourse.tile import TileContext

    f32 = mybir.dt.float32
    nc = bacc.Bacc("TRN2", target_bir_lowering=False)
    strips = nc.dram_tensor("strips", [128, _PTW], f32, kind="ExternalInput")
    bvec = nc.dram_tensor("bvec", [RPC], f32, kind="ExternalInput")
    w_out = nc.dram_tensor("w_out", [RPC, N], f32, kind="ExternalOutput")
    b_out = nc.dram_tensor("b_out", [RPC], f32, kind="ExternalOutput")

    # patch table loaded in block-range chunks; small first chunk => the
    # pipeline starts almost immediately
    qlo = [0, 2, 7, 14, 20, NBLK]
    qcol = [_PCOL[(b, 0)] if b < NBLK else _PTW for b in qlo]
    nq = len(qlo) - 1

    with TileContext(nc) as tc:
        with tc.tile_pool(name="bufs", bufs=1) as pool:
            bufs = [
                pool.tile([128, THIRD], f32, name=f"buf{n}", tag=f"buf{n}")
                for n in range(NBUF)
            ]
            patches = [
                pool.tile(
                    [128, qcol[q + 1] - qcol[q]], f32,
                    name=f"patch{q}", tag=f"patch{q}",
                )
                for q in range(nq)
            ]
            # resident patch table: chunked bulk loads (ACT ring)
            for q in range(nq):
                nc.scalar.dma_start(
                    out=patches[q][:, :],
                    in_=strips[:, qcol[q]:qcol[q + 1]],
                )
            # write buffers start all-zero (split across DVE / GpSimd)
            for n, b in enumerate(bufs):
                eng = nc.vector if n % 2 == 0 else nc.gpsimd
                eng.memset(b[:], 0.0)
            # bias: plain DRAM->DRAM copy (4.6 KB)
            nc.scalar.dma_start(out=b_out[:], in_=bvec[:])
            for T in range(NBLK):
                buf = bufs[T % NBUF]
                q = next(qq for qq in range(nq) if qlo[qq] <= T < qlo[qq + 1])
                pt = patches[q]
                wins = _windows(T)
                # stitch strip windows into the otherwise-zero tile (DVE)
                for wi, (p0w, p1w, lo, width, _) in enumerate(wins):
                    pc = _PCOL[(T, wi)] - qcol[q]
                    nc.vector.tensor_copy(
                        out=buf[p0w:p1w, lo:lo + width],
                        in_=pt[p0w:p1w, pc:pc + width],
                    )
                # one fully-contiguous 1.18 MB write; alternate HWDGE rings
                dst = AP(w_out, T * 128 * THIRD, [[1, 128 * THIRD]])
                (nc.sync if T % 2 == 0 else nc.scalar).dma_start(
                    out=dst, in_=buf[:, :]
                )
                # restore all-zeros for this tile's next use (GpSimd)
                if T + NBUF < NBLK:
                    for (p0w, p1w, lo, width, _) in wins:
                        nc.gpsimd.memset(buf[p0w:p1w, lo:lo + width], 0.0)

    nc.finalize()
    _PROGRAM = nc
    return nc


# test.py hooks: set TRACE=True before calling kernel() to profile; the
# BassKernelResults of the last run lands in LAST_RESULTS.
TRACE = False
TRACE_KWARGS = {}
LAST_RESULTS = None


def kernel(**inputs):
    from concourse.bass_utils import run_bass_kernel_spmd

    kern = np.asarray(inputs["kernel"], dtype=np.float32)
    bias = np.asarray(inputs["bias"], dtype=np.float32)
    stride = int(inputs.get("stride", STRIDE))
    padding = int(inputs.get("padding", PAD))
    assert kern.shape == (O, C, KH, KW), kern.shape
    assert stride == STRIDE and padding == PAD, (stride, padding)

    strips = _build_strips(kern)
    in_maps = [
        {
            "strips": strips[core],
            "bvec": np.repeat(bias[2 * core:2 * core + 2], Ho * Wo),
        }
        for core in range(NCORES)
    ]
    for m in in_maps:
        assert m["bvec"].shape == (RPC,), m["bvec"].shape

    nc = _get_program()
    res = run_bass_kernel_spmd(
        nc,
        in_maps,
        core_ids=list(range(NCORES)),
        trace=TRACE,
        **TRACE_KWARGS,
    )
    global LAST_RESULTS
    LAST_RESULTS = res

    Wm = np.concatenate([res.results[c]["w_out"] for c in range(NCORES)], axis=0)
    bm = np.concatenate([res.results[c]["b_out"] for c in range(NCORES)], axis=0)
    return Wm, bm
